# revision 15
# baseline (speedup 1.0000x reference)
"""Contrastive loss (InfoNCE-style) on 8 Trainium2 NeuronCores.

Reference math (B=8192, D=128, temp=0.07):
    sim = (emb @ emb.T) / temp, diag masked to -1e9
    log_probs = log_softmax(sim, axis=1)
    row_mean_i = mean over positives (same label, j != i) of log_probs[i, :]
    loss = -sum(row_mean_i) / count(rows with >=1 positive)

Decomposition (only the O(B^2) esum runs on device):
    log_probs[i, j] = sim[i, j] - lse_i,  lse_i = log(sum_{j!=i} exp(sim[i, j]))
    pos_sum_i = q_i - pc_i * lse_i with q_i, pc_i exact on host (f64).

Device kernel (SPMD-uniform, no per-core program divergence):
    input  eshard [128, 1024] fp8e4m3 = this core's 1024 columns of emb.T
    - AllGather the 8 shards HBM->HBM -> gath [1024, 1024] (block c =
      core c's [128, 1024] shard), then one 3D-AP DMA -> SBUF embT
      [128, 8192].
    - diag pass: sq = Square(eshard) f32; per row-tile t a [128,1]-moving
      matmul with a ones vector column-sums sq -> ||e_row||^2 in PSUM;
      Exp(x/temp) -> out cols 8:16. This reproduces the in-matmul diag
      term exp(sim_ii/temp) to ~fp32 rounding, so the host can subtract
      it exactly - no diagonal masking (and hence no column rotation or
      per-core mask data) is needed on device.
    - main loop: 8 row-tiles x 4 PSUM quarters x 4 matmuls (fp8, N=512)
      lhs = own shard tile, rhs = gathered embT chunk; scalar-engine
      Exp(x/temp) with f32 accum_out -> esum quarters, reduced 4->1 on
      the ACT engine -> out cols 0:8.
    output outall [128, 16] f32: cols 0:8 esum row-sums (p, t),
    cols 8:16 exp(sim_ii/temp) (p, t).

Host: lse = log(esum - diagexp); row means and the final scalar
reduction in f64 exactly as the reference; the exact q/pc positive
terms are computed from the f32 embeddings (overlapped with the device
round trip, which dominates the wall clock through the axon tunnel).

fp8e4m3 embeddings perturb off-diag sim by ~8e-3 (abs), i.e. lse by
~6e-4 after row averaging: ~100x inside the 2e-2 gate (measured rel err
1.1e-4). The diag term exp(1/temp) ~ 1.6e6 dwarfs the true esum ~
1.8e4 and its fp8 perturbation is ~100x the signal, which is why it is
cancelled with the device-computed value (same fp8 inputs, same fp32
accumulation, same ACT exp) rather than any host-side exp.
"""

import numpy as np

import jax
from jax.sharding import Mesh, PartitionSpec
from jax.experimental.shard_map import shard_map

import concourse.bass as bass
import concourse.mybir as mybir
import concourse.tile as tile
from concourse.tile import add_dep_helper

TEMP = 0.07
B = 8192
D = 128
NCORES = 8
RPC = B // NCORES        # 1024 rows per core
NT = RPC // 128          # 8 row-tiles of 128 rows per core
NQ = 4                   # 4 PSUM quarters of 2048 columns
OUTW = 2 * NT            # 8 esum row-sums + 8 diag exps

_CACHE = {}

# test.py introspection hook (unused by this runner; kept for compat).
last_results = None


def _build_bass():
    f32 = mybir.dt.float32
    bf16 = mybir.dt.bfloat16
    fp8 = mybir.dt.float8e4
    nc = bass.Bass("TRN2", target_bir_lowering=False, debug=False,
                   num_devices=NCORES)
    eshard = nc.dram_tensor("eshard", [128, RPC], fp8, kind="ExternalInput")
    outall = nc.dram_tensor("outall", [128, OUTW], f32, kind="ExternalOutput")

    with tile.TileContext(nc) as tc:
        with (
            tc.tile_pool(name="big", bufs=1) as big,
            tc.tile_pool(name="psum", bufs=2, space="PSUM") as psum,
            tc.tile_pool(name="scratch", bufs=32) as scratch,
            tc.tile_pool(name="small", bufs=1) as small,
            tc.tile_pool(name="dram", bufs=1, space="DRAM") as dram,
        ):
            es = big.tile([128, RPC], fp8)
            nc.sync.dma_start(out=es[:, :], in_=eshard.ap()[:, :])
            es_dma = nc.cur_bb.bb.instructions[-1]

            ones = small.tile([128, 1], f32)
            nc.gpsimd.memset(ones[:, :], 1.0)

            # HBM bounce -> AllGather: gath block c = core c's shard.
            inb = dram.tile([128, RPC], fp8)
            gath = dram.tile([NCORES * 128, RPC], fp8)
            nc.gpsimd.dma_start(inb[:, :], eshard.ap()[:, :])
            bounce_dma = nc.cur_bb.bb.instructions[-1]
            nc.gpsimd.collective_compute(
                "AllGather", mybir.AluOpType.bypass,
                replica_groups=[list(range(NCORES))],
                ins=[inb.opt()], outs=[gath.opt()],
            )
            cc_inst = nc.cur_bb.bb.instructions[-1]

            # one 3D-AP DMA moves all 8 gathered blocks into SBUF
            # (embT[p, c*1024+j] = gath[c*128+p, j]); a single DMA keeps the
            # SP queue count low enough for walrus's wait-limited kernel-tail
            # drain (8 separate DMAs reproducibly overflow it)
            embT = big.tile([128, B], fp8)
            nc.sync.dma_start(
                out=embT[:, :].rearrange("p (c j) -> p c j", c=NCORES),
                in_=gath[:, :].rearrange("(c p) j -> p c j", p=128))
            gather_dma = nc.cur_bb.bb.instructions[-1]

            # manual single-wait drains per input queue / collective, so the
            # wait-limited kernel-tail drain has nothing left to observe
            for dep in (es_dma, bounce_dma, cc_inst, gather_dma):
                nc.sync.drain()
                add_dep_helper(nc.cur_bb.bb.instructions[-1], dep, sync=True,
                               reason="observe producer on SP")

            # prefetch dummies: a discarded LDWEIGHTS per semaphore the PE
            # must observe, so real matmuls carry at most one sync wait
            nc.tensor.ldweights(es[:, 0:2].bitcast(bf16))
            nc.tensor.ldweights(embT[:, 0:2].bitcast(bf16))
            nc.tensor.ldweights(ones[:, :].bitcast(bf16))

            esum_all = small.tile([128, NT * NQ], f32)
            esums_s = small.tile([128, OUTW], f32)

            # diag pass: sq = e^2 (f32), column sums via ones-moving matmuls,
            # exp(x/temp) -> esums_s[:, 8:16]
            sq = big.tile([128, RPC], f32)
            nc.scalar.activation(sq[:, :], es[:, :],
                                 mybir.ActivationFunctionType.Square)
            nc.tensor.ldweights(sq[:, 0:1].bitcast(bf16))
            pd = psum.tile([128, 2048], f32, tag="ps")
            for t in range(NT):
                nc.tensor.matmul(
                    pd[:, t:t + 1],
                    sq[:, t * 128:(t + 1) * 128],
                    ones[:, :],
                    start=True, stop=True,
                )
            nc.scalar.activation(
                esums_s[:, NT:2 * NT], pd[:, 0:NT],
                mybir.ActivationFunctionType.Exp,
                scale=1.0 / TEMP,
            )

            # main loop: esum quarters
            for t in range(NT):
                lhs = es[:, t * 128:(t + 1) * 128]
                for q in range(NQ):
                    qi = t * NQ + q
                    ps = psum.tile([128, 2048], f32, tag="ps")
                    # discarded LDWEIGHTS reading the ACT output that retired
                    # this PSUM slot: carries the ACT wait so the slot-reuse
                    # matmul below carries only its PE wait
                    if qi == 1:
                        nc.tensor.ldweights(
                            esums_s[:, NT:NT + 1].bitcast(bf16))
                        carrier = nc.cur_bb.bb.instructions[-1]
                    elif qi >= 2:
                        nc.tensor.ldweights(
                            esum_all[:, qi - 2:qi - 1].bitcast(bf16))
                        carrier = nc.cur_bb.bb.instructions[-1]
                    else:
                        carrier = None
                    for k in range(4):
                        n = NQ * q + k
                        nc.tensor.matmul(
                            ps[:, k * 512:(k + 1) * 512],
                            lhs,
                            embT[:, n * 512:(n + 1) * 512],
                            start=True, stop=True,
                        )
                        if carrier is not None:
                            add_dep_helper(nc.cur_bb.bb.instructions[-1],
                                           carrier, sync=False,
                                           reason="wait-carrier order")
                            carrier = None
                        last_mm = nc.cur_bb.bb.instructions[-1]
                    scr = scratch.tile([128, 2048], mybir.dt.bfloat16)
                    nc.scalar.activation(
                        scr[:, :], ps[:, :],
                        mybir.ActivationFunctionType.Exp,
                        scale=1.0 / TEMP,
                        accum_out=esum_all[:, qi:qi + 1],
                    )
            # reduce the 4 quarters per row-tile to a single f32 row sum on
            # the ACT engine (in-order wrt the accum writes above, so no
            # extra semaphores)
            junk = small.tile([128, NT * NQ], f32)
            for t in range(NT):
                nc.scalar.activation(
                    junk[:, t * NQ:(t + 1) * NQ],
                    esum_all[:, t * NQ:(t + 1) * NQ],
                    mybir.ActivationFunctionType.Copy,
                    accum_out=esums_s[:, t:t + 1],
                )
            # single-writer funnel: gives the out DMA exactly one producer
            out_s = small.tile([128, OUTW], f32)
            nc.scalar.activation(out_s[:, :], esums_s[:, :],
                                 mybir.ActivationFunctionType.Copy)
            last_act = nc.cur_bb.bb.instructions[-1]

            nc.sync.drain()
            add_dep_helper(nc.cur_bb.bb.instructions[-1], last_mm, sync=True,
                           reason="observe PE on SP")
            nc.sync.drain()
            add_dep_helper(nc.cur_bb.bb.instructions[-1], last_act, sync=True,
                           reason="observe ACT on SP")
            # issue the out DMA from the ACT engine: it directly follows the
            # ACT Copy in the same stream, so it needs no sync waits at all
            nc.scalar.dma_start(out=outall.ap()[:, :], in_=out_s[:, :])
            out_dma = nc.cur_bb.bb.instructions[-1]
            nc.sync.drain()
            add_dep_helper(nc.cur_bb.bb.instructions[-1], out_dma, sync=True,
                           reason="observe out DMA queue on SP")
    return nc


def _get_runner():
    """Build the Bass module once and return a cached jitted SPMD callable.

    Replicates concourse.bass2jax.run_bass_via_pjrt but keeps the jitted
    function across calls: re-tracing + re-lowering per call costs hundreds
    of ms through the axon tunnel, far more than this kernel's transfers.
    """
    if "runner" in _CACHE:
        return _CACHE["runner"]

    from concourse.bass2jax import (
        _bass_exec_p, install_neuronx_cc_hook, partition_id_tensor,
    )

    nc = _build_bass()
    install_neuronx_cc_hook()

    partition_name = (nc.partition_id_tensor.name
                      if nc.partition_id_tensor else None)
    in_names, out_names, out_avals, zero_shapes = [], [], [], []
    for alloc in nc.m.functions[0].allocations:
        if not isinstance(alloc, mybir.MemoryLocationSet):
            continue
        name = alloc.memorylocations[0].name
        if alloc.kind == "ExternalInput":
            if name != partition_name:
                in_names.append(name)
        elif alloc.kind == "ExternalOutput":
            shape = tuple(alloc.tensor_shape)
            dtype = mybir.dt.np(alloc.dtype)
            out_names.append(name)
            out_avals.append(jax.core.ShapedArray(shape, dtype))
            zero_shapes.append((shape, dtype))
    n_params = len(in_names)
    n_outs = len(out_names)
    in_names_all = list(in_names) + list(out_names)
    if partition_name is not None:
        in_names_all.append(partition_name)
    donate = tuple(range(n_params, n_params + n_outs))

    def _body(*args):
        operands = list(args)
        if partition_name is not None:
            operands.append(partition_id_tensor())
        outs = _bass_exec_p.bind(
            *operands,
            out_avals=tuple(out_avals),
            in_names=tuple(in_names_all),
            out_names=tuple(out_names),
            lowering_input_output_aliases=(),
            sim_require_finite=True,
            sim_require_nnan=True,
            nc=nc,
        )
        return tuple(outs)

    devices = jax.devices()[:NCORES]
    assert len(devices) == NCORES, (
        f"need {NCORES} devices, found {len(jax.devices())}")
    mesh = Mesh(np.asarray(devices), ("core",))
    sharded = jax.jit(
        shard_map(_body, mesh=mesh,
                  in_specs=(PartitionSpec("core"),) * (n_params + n_outs),
                  out_specs=(PartitionSpec("core"),) * n_outs,
                  check_rep=False),
        keep_unused=True,
    )
    # The "output" operands only exist because run_neff-style kernels may
    # rely on pre-zeroed output buffers; this kernel writes every element
    # and the custom call produces fresh result buffers (no aliasing), so
    # park the zeros on device once and never re-upload them.
    from jax.sharding import NamedSharding
    zsh = NamedSharding(mesh, PartitionSpec("core"))
    dev_zeros = [
        jax.device_put(np.zeros((NCORES * s[0], *s[1:]), dt), zsh)
        for s, dt in zero_shapes
    ]
    jax.block_until_ready(dev_zeros)
    _CACHE["runner"] = (sharded, in_names, dev_zeros)
    return _CACHE["runner"]


def kernel(embeddings, labels):
    emb = np.asarray(embeddings, dtype=np.float32)
    labels = np.asarray(labels).astype(np.int64)
    assert emb.shape == (B, D) and labels.shape == (B,)

    sharded, in_names, dev_zeros = _get_runner()

    f8 = mybir.dt.np(mybir.dt.float8e4)
    # eshard_cat[c*128 + d, j] = fp8(emb[c*1024 + j, d]) in one strided pass
    eshard_cat = (emb.reshape(NCORES, RPC, D).transpose(0, 2, 1)
                  .astype(f8).reshape(NCORES * 128, RPC))

    # async dispatch: upload + exec + (eager) fetch run while the host
    # computes the exact q/pc terms below
    out_arrs = sharded(eshard_cat, *dev_zeros)
    try:
        out_arrs[0].copy_to_host_async()
    except Exception:
        pass

    emb64 = emb.astype(np.float64)
    nclass = int(labels.max()) + 1
    cnt = np.bincount(labels, minlength=nclass)
    pc = cnt[labels] - 1                      # positives per row (excl. self)
    G = np.zeros((nclass, D), dtype=np.float64)
    np.add.at(G, labels, emb64)
    # q_i = sum over positives j (same label, j != i) of sim[i, j]
    q = (np.einsum("ij,ij->i", emb64, G[labels])
         - np.einsum("ij,ij->i", emb64, emb64)) / TEMP
    has = pc > 0

    oa = np.asarray(out_arrs[0]).reshape(NCORES, 128, OUTW).astype(np.float64)
    # outall[c, p, t] -> esum row sum; outall[c, p, 8+t] -> diag exp.
    # local row j = 128t + p, global row = 1024c + j.
    esum = oa[:, :, :NT].transpose(0, 2, 1).reshape(-1)       # [B]
    dexp = oa[:, :, NT:].transpose(0, 2, 1).reshape(-1)       # [B]
    lse = np.log(esum - dexp)

    row_mean = np.where(has, q / np.maximum(pc, 1) - lse, 0.0)
    loss = -row_mean.sum() / max(int(has.sum()), 1)
    return np.float32(loss)


# revision 16
# speedup vs baseline: 1.1373x; 1.1373x over previous
"""Contrastive loss (InfoNCE-style) on 8 Trainium2 NeuronCores.

Reference math (B=8192, D=128, temp=0.07):
    sim = (emb @ emb.T) / temp, diag masked to -1e9
    log_probs = log_softmax(sim, axis=1)
    row_mean_i = mean over positives (same label, j != i) of log_probs[i, :]
    loss = -sum(row_mean_i) / count(rows with >=1 positive)

Decomposition (only the O(B^2) esum runs on device):
    log_probs[i, j] = sim[i, j] - lse_i,  lse_i = log(sum_{j!=i} exp(sim[i, j]))
    pos_sum_i = q_i - pc_i * lse_i with q_i, pc_i exact on host (f64).

Device kernel (SPMD-uniform, no per-core program divergence):
    input  eshard [128, 1024] fp8e4m3 = this core's 1024 columns of emb.T
    - AllGather the 8 shards HBM->HBM -> gath [1024, 1024] (block c =
      core c's [128, 1024] shard), then one 3D-AP DMA -> SBUF embT
      [128, 8192].
    - diag pass: sq = Square(eshard) f32; per row-tile t a [128,1]-moving
      matmul with a ones vector column-sums sq -> ||e_row||^2 in PSUM;
      Exp(x/temp) -> out cols 8:16. This reproduces the in-matmul diag
      term exp(sim_ii/temp) to ~fp32 rounding, so the host can subtract
      it exactly - no diagonal masking (and hence no column rotation or
      per-core mask data) is needed on device.
    - main loop: 8 row-tiles x 4 PSUM quarters x 4 matmuls (fp8, N=512)
      lhs = own shard tile, rhs = gathered embT chunk; scalar-engine
      Exp(x/temp) with f32 accum_out -> esum quarters, reduced 4->1 on
      the ACT engine -> out cols 0:8.
    output outall [128, 16] f32: cols 0:8 esum row-sums (p, t),
    cols 8:16 exp(sim_ii/temp) (p, t).

Host: lse = log(esum - diagexp); row means and the final scalar
reduction in f64 exactly as the reference; the exact q/pc positive
terms are computed from the f32 embeddings (overlapped with the device
round trip, which dominates the wall clock through the axon tunnel).

fp8e4m3 embeddings perturb off-diag sim by ~8e-3 (abs), i.e. lse by
~6e-4 after row averaging: ~100x inside the 2e-2 gate (measured rel err
1.1e-4). The diag term exp(1/temp) ~ 1.6e6 dwarfs the true esum ~
1.8e4 and its fp8 perturbation is ~100x the signal, which is why it is
cancelled with the device-computed value (same fp8 inputs, same fp32
accumulation, same ACT exp) rather than any host-side exp.
"""

import numpy as np

import jax
from jax.sharding import Mesh, PartitionSpec
from jax.experimental.shard_map import shard_map

import concourse.bass as bass
import concourse.mybir as mybir
import concourse.tile as tile
from concourse.tile import add_dep_helper

TEMP = 0.07
B = 8192
D = 128
NCORES = 8
RPC = B // NCORES        # 1024 rows per core
NT = RPC // 128          # 8 row-tiles of 128 rows per core
NQ = 4                   # 4 PSUM quarters of 2048 columns
OUTW = 2 * NT            # 8 esum row-sums + 8 diag exps

_CACHE = {}

# test.py introspection hook (unused by this runner; kept for compat).
last_results = None


def _build_bass():
    f32 = mybir.dt.float32
    bf16 = mybir.dt.bfloat16
    fp8 = mybir.dt.float8e4
    nc = bass.Bass("TRN2", target_bir_lowering=False, debug=False,
                   num_devices=NCORES)
    eshard = nc.dram_tensor("eshard", [128, RPC], fp8, kind="ExternalInput")
    outall = nc.dram_tensor("outall", [128, OUTW], f32, kind="ExternalOutput")

    with tile.TileContext(nc) as tc:
        with (
            tc.tile_pool(name="big", bufs=1) as big,
            tc.tile_pool(name="psum", bufs=2, space="PSUM") as psum,
            tc.tile_pool(name="scratch", bufs=32) as scratch,
            tc.tile_pool(name="small", bufs=1) as small,
            tc.tile_pool(name="dram", bufs=1, space="DRAM") as dram,
        ):
            es = big.tile([128, RPC], fp8)
            nc.sync.dma_start(out=es[:, :], in_=eshard.ap()[:, :])
            es_dma = nc.cur_bb.bb.instructions[-1]

            ones = small.tile([128, 1], f32)
            nc.gpsimd.memset(ones[:, :], 1.0)

            # HBM bounce -> AllGather: gath block c = core c's shard.
            inb = dram.tile([128, RPC], fp8)
            gath = dram.tile([NCORES * 128, RPC], fp8)
            nc.gpsimd.dma_start(inb[:, :], eshard.ap()[:, :])
            bounce_dma = nc.cur_bb.bb.instructions[-1]
            nc.gpsimd.collective_compute(
                "AllGather", mybir.AluOpType.bypass,
                replica_groups=[list(range(NCORES))],
                ins=[inb.opt()], outs=[gath.opt()],
            )
            cc_inst = nc.cur_bb.bb.instructions[-1]

            # one 3D-AP DMA moves all 8 gathered blocks into SBUF
            # (embT[p, c*1024+j] = gath[c*128+p, j]); a single DMA keeps the
            # SP queue count low enough for walrus's wait-limited kernel-tail
            # drain (8 separate DMAs reproducibly overflow it)
            embT = big.tile([128, B], fp8)
            nc.sync.dma_start(
                out=embT[:, :].rearrange("p (c j) -> p c j", c=NCORES),
                in_=gath[:, :].rearrange("(c p) j -> p c j", p=128))
            gather_dma = nc.cur_bb.bb.instructions[-1]

            # manual single-wait drains per input queue / collective, so the
            # wait-limited kernel-tail drain has nothing left to observe
            for dep in (es_dma, bounce_dma, cc_inst, gather_dma):
                nc.sync.drain()
                add_dep_helper(nc.cur_bb.bb.instructions[-1], dep, sync=True,
                               reason="observe producer on SP")

            # prefetch dummies: a discarded LDWEIGHTS per semaphore the PE
            # must observe, so real matmuls carry at most one sync wait
            nc.tensor.ldweights(es[:, 0:2].bitcast(bf16))
            nc.tensor.ldweights(embT[:, 0:2].bitcast(bf16))
            nc.tensor.ldweights(ones[:, :].bitcast(bf16))

            esum_all = small.tile([128, NT * NQ], f32)
            esums_s = small.tile([128, OUTW], f32)

            # diag pass: sq = e^2 (f32), column sums via ones-moving matmuls,
            # exp(x/temp) -> esums_s[:, 8:16]
            sq = big.tile([128, RPC], f32)
            nc.scalar.activation(sq[:, :], es[:, :],
                                 mybir.ActivationFunctionType.Square)
            nc.tensor.ldweights(sq[:, 0:1].bitcast(bf16))
            pd = psum.tile([128, 2048], f32, tag="ps")
            for t in range(NT):
                nc.tensor.matmul(
                    pd[:, t:t + 1],
                    sq[:, t * 128:(t + 1) * 128],
                    ones[:, :],
                    start=True, stop=True,
                )
            nc.scalar.activation(
                esums_s[:, NT:2 * NT], pd[:, 0:NT],
                mybir.ActivationFunctionType.Exp,
                scale=1.0 / TEMP,
            )

            # main loop: esum quarters
            for t in range(NT):
                lhs = es[:, t * 128:(t + 1) * 128]
                for q in range(NQ):
                    qi = t * NQ + q
                    ps = psum.tile([128, 2048], f32, tag="ps")
                    # discarded LDWEIGHTS reading the ACT output that retired
                    # this PSUM slot: carries the ACT wait so the slot-reuse
                    # matmul below carries only its PE wait
                    if qi == 1:
                        nc.tensor.ldweights(
                            esums_s[:, NT:NT + 1].bitcast(bf16))
                        carrier = nc.cur_bb.bb.instructions[-1]
                    elif qi >= 2:
                        nc.tensor.ldweights(
                            esum_all[:, qi - 2:qi - 1].bitcast(bf16))
                        carrier = nc.cur_bb.bb.instructions[-1]
                    else:
                        carrier = None
                    for k in range(4):
                        n = NQ * q + k
                        nc.tensor.matmul(
                            ps[:, k * 512:(k + 1) * 512],
                            lhs,
                            embT[:, n * 512:(n + 1) * 512],
                            start=True, stop=True,
                        )
                        if carrier is not None:
                            add_dep_helper(nc.cur_bb.bb.instructions[-1],
                                           carrier, sync=False,
                                           reason="wait-carrier order")
                            carrier = None
                        last_mm = nc.cur_bb.bb.instructions[-1]
                    scr = scratch.tile([128, 2048], mybir.dt.bfloat16)
                    nc.scalar.activation(
                        scr[:, :], ps[:, :],
                        mybir.ActivationFunctionType.Exp,
                        scale=1.0 / TEMP,
                        accum_out=esum_all[:, qi:qi + 1],
                    )
            # reduce the 4 quarters per row-tile to a single f32 row sum on
            # the ACT engine (in-order wrt the accum writes above, so no
            # extra semaphores)
            junk = small.tile([128, NT * NQ], f32)
            for t in range(NT):
                nc.scalar.activation(
                    junk[:, t * NQ:(t + 1) * NQ],
                    esum_all[:, t * NQ:(t + 1) * NQ],
                    mybir.ActivationFunctionType.Copy,
                    accum_out=esums_s[:, t:t + 1],
                )
            # single-writer funnel: gives the out DMA exactly one producer
            out_s = small.tile([128, OUTW], f32)
            nc.scalar.activation(out_s[:, :], esums_s[:, :],
                                 mybir.ActivationFunctionType.Copy)
            last_act = nc.cur_bb.bb.instructions[-1]

            nc.sync.drain()
            add_dep_helper(nc.cur_bb.bb.instructions[-1], last_mm, sync=True,
                           reason="observe PE on SP")
            nc.sync.drain()
            add_dep_helper(nc.cur_bb.bb.instructions[-1], last_act, sync=True,
                           reason="observe ACT on SP")
            # issue the out DMA from the ACT engine: it directly follows the
            # ACT Copy in the same stream, so it needs no sync waits at all
            nc.scalar.dma_start(out=outall.ap()[:, :], in_=out_s[:, :])
            out_dma = nc.cur_bb.bb.instructions[-1]
            nc.sync.drain()
            add_dep_helper(nc.cur_bb.bb.instructions[-1], out_dma, sync=True,
                           reason="observe out DMA queue on SP")
    return nc


def _get_runner():
    """Build the Bass module once and return a cached jitted SPMD callable.

    Replicates concourse.bass2jax.run_bass_via_pjrt but keeps the jitted
    function across calls: re-tracing + re-lowering per call costs hundreds
    of ms through the axon tunnel, far more than this kernel's transfers.
    """
    if "runner" in _CACHE:
        return _CACHE["runner"]

    from concourse.bass2jax import (
        _bass_exec_p, install_neuronx_cc_hook, partition_id_tensor,
    )

    nc = _build_bass()
    install_neuronx_cc_hook()

    partition_name = (nc.partition_id_tensor.name
                      if nc.partition_id_tensor else None)
    in_names, out_names, out_avals, zero_shapes = [], [], [], []
    for alloc in nc.m.functions[0].allocations:
        if not isinstance(alloc, mybir.MemoryLocationSet):
            continue
        name = alloc.memorylocations[0].name
        if alloc.kind == "ExternalInput":
            if name != partition_name:
                in_names.append(name)
        elif alloc.kind == "ExternalOutput":
            shape = tuple(alloc.tensor_shape)
            dtype = mybir.dt.np(alloc.dtype)
            out_names.append(name)
            out_avals.append(jax.core.ShapedArray(shape, dtype))
            zero_shapes.append((shape, dtype))
    n_params = len(in_names)
    n_outs = len(out_names)
    in_names_all = list(in_names) + list(out_names)
    if partition_name is not None:
        in_names_all.append(partition_name)
    donate = tuple(range(n_params, n_params + n_outs))

    def _body(*args):
        operands = list(args)
        if partition_name is not None:
            operands.append(partition_id_tensor())
        outs = _bass_exec_p.bind(
            *operands,
            out_avals=tuple(out_avals),
            in_names=tuple(in_names_all),
            out_names=tuple(out_names),
            lowering_input_output_aliases=(),
            sim_require_finite=True,
            sim_require_nnan=True,
            nc=nc,
        )
        return tuple(outs)

    devices = jax.devices()[:NCORES]
    assert len(devices) == NCORES, (
        f"need {NCORES} devices, found {len(jax.devices())}")
    mesh = Mesh(np.asarray(devices), ("core",))
    sharded = jax.jit(
        shard_map(_body, mesh=mesh,
                  in_specs=(PartitionSpec("core"),) * (n_params + n_outs),
                  out_specs=(PartitionSpec("core"),) * n_outs,
                  check_rep=False),
        keep_unused=True,
    )
    # The "output" operands only exist because run_neff-style kernels may
    # rely on pre-zeroed output buffers; this kernel writes every element
    # and the custom call produces fresh result buffers (no aliasing), so
    # park the zeros on device once and never re-upload them.
    from jax.sharding import NamedSharding
    zsh = NamedSharding(mesh, PartitionSpec("core"))
    dev_zeros = [
        jax.device_put(np.zeros((NCORES * s[0], *s[1:]), dt), zsh)
        for s, dt in zero_shapes
    ]
    jax.block_until_ready(dev_zeros)
    _CACHE["runner"] = (sharded, in_names, dev_zeros)
    return _CACHE["runner"]


def kernel(embeddings, labels):
    emb = np.asarray(embeddings, dtype=np.float32)
    labels = np.asarray(labels).astype(np.int64)
    assert emb.shape == (B, D) and labels.shape == (B,)

    sharded, in_names, dev_zeros = _get_runner()

    f8 = mybir.dt.np(mybir.dt.float8e4)
    # eshard_cat[c*128 + d, j] = fp8(emb[c*1024 + j, d]) in one strided pass
    eshard_cat = (emb.reshape(NCORES, RPC, D).transpose(0, 2, 1)
                  .astype(f8).reshape(NCORES * 128, RPC))

    # async dispatch: upload + exec + (eager) fetch run while the host
    # computes the exact q/pc terms below
    out_arrs = sharded(eshard_cat, *dev_zeros)

    emb64 = emb.astype(np.float64)
    nclass = int(labels.max()) + 1
    cnt = np.bincount(labels, minlength=nclass)
    pc = cnt[labels] - 1                      # positives per row (excl. self)
    G = np.zeros((nclass, D), dtype=np.float64)
    np.add.at(G, labels, emb64)
    # q_i = sum over positives j (same label, j != i) of sim[i, j]
    q = (np.einsum("ij,ij->i", emb64, G[labels])
         - np.einsum("ij,ij->i", emb64, emb64)) / TEMP
    has = pc > 0

    oa = np.asarray(out_arrs[0]).reshape(NCORES, 128, OUTW).astype(np.float64)
    # outall[c, p, t] -> esum row sum; outall[c, p, 8+t] -> diag exp.
    # local row j = 128t + p, global row = 1024c + j.
    esum = oa[:, :, :NT].transpose(0, 2, 1).reshape(-1)       # [B]
    dexp = oa[:, :, NT:].transpose(0, 2, 1).reshape(-1)       # [B]
    lse = np.log(esum - dexp)

    row_mean = np.where(has, q / np.maximum(pc, 1) - lse, 0.0)
    loss = -row_mean.sum() / max(int(has.sum()), 1)
    return np.float32(loss)
